# revision 5
# baseline (speedup 1.0000x reference)
"""Trainium2 Bass kernel for nn_DenseDilatedKnnGraph.

Contract: kernel(x, code) takes FULL inputs (numpy), returns the FULL output
(edge_index int32 (2, 2, 4096, 9), drl f32 scalar) exactly like the jax
reference.

Sharding: 8 cores = 2 batches x 4 row-blocks of 1024 query rows. Every core
receives the full key set of its batch plus its query slice (prepared on the
host), computes its 1024x4096 similarity tiles on the PE, and reduces to
per-row outputs (top-k indices, argmin/argmax, loss row-sums) on-chip. Only
O(N) data ever leaves the device.

Math notes (u = "negated distance before relu"):
  ux[i,j] = 2*Sx - ax_i - ax_j   (dist_x = relu(-ux))
  uc[i,j] = 2*Sc - ac_i - ac_j   (dist_c = relu(-uc))
  m = ux + uc                    (fused ranking value; top-k of dist = max8 of m)
  A[i] = sum_j uc*ux  ~= sum_j max(dist_c,1e-5)*dist_x   (loss row-sums; the
         1e-5 clamp and relu only matter on the diagonal, ~1e-12 effect)
  B[i] = -sum_j uc    ~= sum_j max(dist_c,1e-5)
The distance matrices are symmetric, so row sums equal the column sums the
reference's gathered-matrix means reduce to.
"""
import sys

if "/opt/trn_rl_repo" not in sys.path:
    sys.path.insert(0, "/opt/trn_rl_repo")

import numpy as np

B = 2
N = 4096
CX = 192
CC = 64
ROWS_PER_CORE = 1024
QT = ROWS_PER_CORE // 128        # query tiles per core
KC = N // 512                    # key chunks per row tile
W = 256                          # top-k candidate window
NWIN = N // W                    # 16 windows -> 128 candidates
OUTW = 48                        # packed output width per row
EPS = np.float32(1e-12)

_PROGRAM = None


def _build_program():
    import concourse.mybir as mybir
    import concourse.tile as tile
    from concourse import bacc

    f32 = mybir.dt.float32
    u32 = mybir.dt.uint32
    AF = mybir.ActivationFunctionType
    OP = mybir.AluOpType

    nc = bacc.Bacc(None, target_bir_lowering=False, debug=False)

    xk0_d = nc.dram_tensor("xk0", [128, N], f32, kind="ExternalInput").ap()
    xk1_d = nc.dram_tensor("xk1", [66, N], f32, kind="ExternalInput").ap()
    ck_d = nc.dram_tensor("ck", [66, N], f32, kind="ExternalInput").ap()
    xq0_d = nc.dram_tensor("xq0", [128, ROWS_PER_CORE], f32, kind="ExternalInput").ap()
    xq1_d = nc.dram_tensor("xq1", [66, ROWS_PER_CORE], f32, kind="ExternalInput").ap()
    cq_d = nc.dram_tensor("cq", [66, ROWS_PER_CORE], f32, kind="ExternalInput").ap()
    out_d = nc.dram_tensor("out", [ROWS_PER_CORE, OUTW], f32, kind="ExternalOutput").ap()

    with tile.TileContext(nc) as tc:
        with (
            tc.tile_pool(name="inp", bufs=1) as inp,
            tc.tile_pool(name="rows", bufs=2) as rows,
            tc.tile_pool(name="chunk", bufs=3) as chunk,
            tc.tile_pool(name="small", bufs=2) as small,
            tc.tile_pool(name="ps", bufs=2, space="PSUM") as ps,
        ):
            xk0 = inp.tile([128, N], f32)
            xk1 = inp.tile([66, N], f32)
            ck = inp.tile([66, N], f32)
            xq0 = inp.tile([128, ROWS_PER_CORE], f32)
            xq1 = inp.tile([66, ROWS_PER_CORE], f32)
            cq = inp.tile([66, ROWS_PER_CORE], f32)
            nc.sync.dma_start(xk0[:], xk0_d[:])
            nc.sync.dma_start(xk1[:], xk1_d[:])
            nc.sync.dma_start(ck[:], ck_d[:])
            nc.sync.dma_start(xq0[:], xq0_d[:])
            nc.sync.dma_start(xq1[:], xq1_d[:])
            nc.sync.dma_start(cq[:], cq_d[:])

            for qt in range(QT):
                qs = slice(qt * 128, (qt + 1) * 128)
                ux_t = rows.tile([128, N], f32, tag="ux")
                dx_t = rows.tile([128, N], f32, tag="dx")
                m_t = rows.tile([128, N], f32, tag="m")
                aparts = small.tile([128, KC], f32, tag="aparts")
                bparts = small.tile([128, KC], f32, tag="bparts")

                for kc in range(KC):
                    ks = slice(kc * 512, (kc + 1) * 512)
                    p2 = ps.tile([128, 512], f32, tag="p2")
                    nc.tensor.matmul(p2[:], xq0[:, qs], xk0[:, ks], start=True, stop=False)
                    nc.tensor.matmul(p2[:], xq1[:, qs], xk1[:, ks], start=False, stop=True)
                    p1 = ps.tile([128, 512], f32, tag="p1")
                    nc.tensor.matmul(p1[:], cq[:, qs], ck[:, ks], start=True, stop=True)

                    # ACT: ux, dx(-ux) from P2; uc from P1 with row-sum accum (B)
                    nc.scalar.copy(ux_t[:, ks], p2[:])
                    nc.scalar.activation(dx_t[:, ks], p2[:], AF.Copy, bias=0.0, scale=-1.0)
                    uc_c = chunk.tile([128, 512], f32, tag="uc")
                    nc.scalar.activation(
                        uc_c[:], p1[:], AF.Copy, bias=0.0, scale=1.0,
                        accum_out=bparts[:, kc:kc + 1],
                    )
                    # GPSIMD: m = ux + uc; t = uc*ux (for the loss row-sums)
                    nc.gpsimd.tensor_tensor(m_t[:, ks], ux_t[:, ks], uc_c[:], op=OP.add)
                    scr = chunk.tile([128, 512], f32, tag="scr")
                    nc.gpsimd.tensor_tensor(scr[:], ux_t[:, ks], uc_c[:], op=OP.mult)
                    # DVE: A partial = sum_j uc*ux
                    nc.vector.tensor_reduce(
                        aparts[:, kc:kc + 1], scr[:],
                        axis=mybir.AxisListType.X, op=OP.add)

                outt = small.tile([128, OUTW], f32, tag="outt")
                # layout: 0:8 IK(u32) | 8:10 ICF(u32) | 10:11 A | 11:12 B |
                #         12:36 V(f32) | 36:48 zeros
                # ---- fused top-k: per-window top-8 candidates, merge, stride-2 index
                cm = small.tile([128, NWIN * 8], f32, tag="cm")
                for w in range(NWIN):
                    nc.vector.max(cm[:, w * 8:(w + 1) * 8], m_t[:, w * W:(w + 1) * W])
                nc.vector.max(outt[:, 12:20], cm[:])                      # ranks 0-7
                nc.vector.match_replace(cm[:], outt[:, 12:20], cm[:], -1.0e30)
                nc.vector.max(outt[:, 20:28], cm[:])                      # ranks 8-15
                nc.vector.match_replace(cm[:], outt[:, 20:28], cm[:], -1.0e30)
                nc.vector.max(outt[:, 28:36], cm[:])                      # ranks 16-23
                # indices of ranks 2,4,...,16 == V cols 14,16,...,28
                nc.vector.max_index(
                    outt[:, 0:8].bitcast(u32), outt[:, 14:30:2], m_t[:])

                # ---- closest (rank-1 of ux max8; rank-0 is the diagonal) and
                # ---- farthest (rank-0 of (-ux) max8), indexed in ux
                vx = small.tile([128, 8], f32, tag="vx")
                nc.vector.max(vx[:], ux_t[:])
                vf = small.tile([128, 8], f32, tag="vf")
                nc.vector.max(vf[:], dx_t[:])
                w2 = small.tile([128, 8], f32, tag="w2")
                nc.vector.memset(w2[:, 2:8], -1.0e30)
                nc.vector.tensor_copy(w2[:, 0:1], vx[:, 1:2])
                nc.vector.tensor_scalar_mul(w2[:, 1:2], vf[:, 0:1], -1.0)
                icf_scr = small.tile([128, 8], u32, tag="icf")
                nc.vector.max_index(icf_scr[:], w2[:], ux_t[:])
                nc.vector.tensor_copy(outt[:, 8:10].bitcast(u32), icf_scr[:, 0:2])

                # ---- A, B row sums
                nc.vector.tensor_reduce(
                    outt[:, 10:11], aparts[:], axis=mybir.AxisListType.X, op=OP.add)
                nc.vector.tensor_reduce(
                    outt[:, 11:12], bparts[:], axis=mybir.AxisListType.X, op=OP.add)

                nc.vector.tensor_copy(outt[:, 36:44], aparts[:])   # debug
                nc.vector.tensor_copy(outt[:, 44:48], bparts[:, 0:4])  # debug
                nc.sync.dma_start(out_d[qs, :], outt[:])

    nc.compile()
    return nc


def _get_program():
    global _PROGRAM
    if _PROGRAM is None:
        _PROGRAM = _build_program()
    return _PROGRAM


def _host_prep(xb, cb):
    """Exact f32 prep for one batch: normalized features + sumsq rows."""
    nx = np.sqrt((xb * xb).sum(axis=0))
    xn = xb / np.maximum(nx, EPS)
    ncn = np.sqrt((cb * cb).sum(axis=0))
    cn = cb / np.maximum(ncn, EPS)
    ax = (xn * xn).sum(axis=0).astype(np.float32)
    ac = (cn * cn).sum(axis=0).astype(np.float32)
    return xn, cn, ax, ac


def _in_map(xn, cn, ax, ac, r0):
    ones_k = np.ones((1, N), np.float32)
    q = slice(r0, r0 + ROWS_PER_CORE)
    neg1_q = np.full((1, ROWS_PER_CORE), -1.0, np.float32)
    return {
        "xk0": np.ascontiguousarray(xn[0:128]),
        "xk1": np.ascontiguousarray(np.concatenate([xn[128:192], ones_k, ax[None]], 0)),
        "ck": np.ascontiguousarray(np.concatenate([cn, ones_k, ac[None]], 0)),
        "xq0": np.ascontiguousarray(2.0 * xn[0:128, q]),
        "xq1": np.ascontiguousarray(
            np.concatenate([2.0 * xn[128:192, q], -ax[None, q], neg1_q], 0)),
        "cq": np.ascontiguousarray(
            np.concatenate([2.0 * cn[:, q], -ac[None, q], neg1_q], 0)),
    }


def kernel(x, code):
    from concourse.bass_utils import run_bass_kernel_spmd

    x = np.asarray(x)
    code = np.asarray(code)
    nc = _get_program()

    in_maps = []
    for b in range(B):
        xn, cn, ax, ac = _host_prep(
            x[b, :, :, 0].astype(np.float32), code[b, :, :, 0].astype(np.float32))
        for rb in range(4):
            in_maps.append(_in_map(xn, cn, ax, ac, rb * ROWS_PER_CORE))

    res = run_bass_kernel_spmd(nc, in_maps, core_ids=list(range(8)))

    edge_nn = np.empty((B, N, 9), np.int32)
    closest = np.empty((B, N), np.int64)
    farthest = np.empty((B, N), np.int64)
    A = np.empty((B, N), np.float64)
    Bv = np.empty((B, N), np.float64)
    for ci in range(8):
        b, rb = divmod(ci, 4)
        r0 = rb * ROWS_PER_CORE
        o = res.results[ci]["out"]
        sl = slice(r0, r0 + ROWS_PER_CORE)
        ik = o[:, 0:8].view(np.uint32).astype(np.int32)
        icf = o[:, 8:10].view(np.uint32)
        edge_nn[b, sl, 0] = np.arange(r0, r0 + ROWS_PER_CORE, dtype=np.int32)
        edge_nn[b, sl, 1:] = ik
        closest[b, sl] = icf[:, 0]
        farthest[b, sl] = icf[:, 1]
        A[b, sl] = o[:, 10].astype(np.float64)
        Bv[b, sl] = -o[:, 11].astype(np.float64)

    tot = 0.0
    for b in range(B):
        s_intra = (-A[b] + 0.12 * Bv[b]).sum()
        cnt_c = np.bincount(closest[b], minlength=N).astype(np.float64)
        cnt_f = np.bincount(farthest[b], minlength=N).astype(np.float64)
        s_pos = (cnt_c * (-A[b] + 0.2 * Bv[b])).sum()
        s_neg = (cnt_f * (-A[b] + 1.0 * Bv[b])).sum()
        tot += 0.1 * s_intra + 1.0 * s_pos + 0.15 * s_neg
    drl = np.float32(tot / (B * N * N))

    center = np.broadcast_to(np.arange(N, dtype=np.int32)[None, :, None], (B, N, 9))
    edge_index = np.stack([edge_nn, np.ascontiguousarray(center)], axis=0).astype(np.int32)
    return edge_index, drl


# revision 7
# speedup vs baseline: 22.6186x; 22.6186x over previous
"""Trainium2 Bass kernel for nn_DenseDilatedKnnGraph.

Contract: kernel(x, code) takes FULL inputs (numpy), returns the FULL output
(edge_index int32 (2, 2, 4096, 9), drl f32 scalar) exactly like the jax
reference.

Sharding: 8 cores = 2 batches x 4 row-blocks of 1024 query rows. Every core
receives the full key set of its batch plus its query slice (prepared on the
host), computes its 1024x4096 similarity tiles on the PE, and reduces to
per-row outputs (top-k indices, argmin/argmax, loss row-sums) on-chip. Only
O(N) data ever leaves the device.

Math notes (u = "negated distance before relu"):
  ux[i,j] = 2*Sx - ax_i - ax_j   (dist_x = relu(-ux))
  uc[i,j] = 2*Sc - ac_i - ac_j   (dist_c = relu(-uc))
  m = ux + uc                    (fused ranking value; top-k of dist = max8 of m)
  A[i] = sum_j uc*ux  ~= sum_j max(dist_c,1e-5)*dist_x   (loss row-sums; the
         1e-5 clamp and relu only matter on the diagonal, ~1e-12 effect)
  B[i] = -sum_j uc    ~= sum_j max(dist_c,1e-5)
The distance matrices are symmetric, so row sums equal the column sums the
reference's gathered-matrix means reduce to.
"""
import sys

if "/opt/trn_rl_repo" not in sys.path:
    sys.path.insert(0, "/opt/trn_rl_repo")

import numpy as np

B = 2
N = 4096
CX = 192
CC = 64
ROWS_PER_CORE = 1024
QT = ROWS_PER_CORE // 128        # query tiles per core
KC = N // 512                    # key chunks per row tile
W = 256                          # top-k candidate window
NWIN = N // W                    # 16 windows -> 128 candidates
OUTW = 48                        # packed output width per row
EPS = np.float32(1e-12)

_PROGRAM = None


def _build_program(reps=1):
    import concourse.mybir as mybir
    import concourse.tile as tile
    from concourse import bacc

    f32 = mybir.dt.float32
    u32 = mybir.dt.uint32
    AF = mybir.ActivationFunctionType
    OP = mybir.AluOpType

    nc = bacc.Bacc(None, target_bir_lowering=False, debug=False)

    xk0_d = nc.dram_tensor("xk0", [128, N], f32, kind="ExternalInput").ap()
    xk1_d = nc.dram_tensor("xk1", [66, N], f32, kind="ExternalInput").ap()
    ck_d = nc.dram_tensor("ck", [66, N], f32, kind="ExternalInput").ap()
    xq0_d = nc.dram_tensor("xq0", [128, ROWS_PER_CORE], f32, kind="ExternalInput").ap()
    xq1_d = nc.dram_tensor("xq1", [66, ROWS_PER_CORE], f32, kind="ExternalInput").ap()
    cq_d = nc.dram_tensor("cq", [66, ROWS_PER_CORE], f32, kind="ExternalInput").ap()
    out_d = nc.dram_tensor("out", [ROWS_PER_CORE, OUTW], f32, kind="ExternalOutput").ap()

    with tile.TileContext(nc) as tc:
        with (
            tc.tile_pool(name="inp", bufs=1) as inp,
            tc.tile_pool(name="rows", bufs=2) as rows,
            tc.tile_pool(name="chunk", bufs=3) as chunk,
            tc.tile_pool(name="small", bufs=2) as small,
            tc.tile_pool(name="ps", bufs=2, space="PSUM") as ps,
        ):
            xk0 = inp.tile([128, N], f32)
            xk1 = inp.tile([66, N], f32)
            ck = inp.tile([66, N], f32)
            xq0 = inp.tile([128, ROWS_PER_CORE], f32)
            xq1 = inp.tile([66, ROWS_PER_CORE], f32)
            cq = inp.tile([66, ROWS_PER_CORE], f32)
            nc.sync.dma_start(xk0[:], xk0_d[:])
            nc.sync.dma_start(xk1[:], xk1_d[:])
            nc.sync.dma_start(ck[:], ck_d[:])
            nc.sync.dma_start(xq0[:], xq0_d[:])
            nc.sync.dma_start(xq1[:], xq1_d[:])
            nc.sync.dma_start(cq[:], cq_d[:])

            for qt_rep in range(QT * reps):
                qt = qt_rep % QT
                qs = slice(qt * 128, (qt + 1) * 128)
                ux_t = rows.tile([128, N], f32, tag="ux")
                dx_t = rows.tile([128, N], f32, tag="dx")
                m_t = rows.tile([128, N], f32, tag="m")
                aparts = small.tile([128, KC], f32, tag="aparts")
                bparts = small.tile([128, KC], f32, tag="bparts")

                for kc in range(KC):
                    ks = slice(kc * 512, (kc + 1) * 512)
                    p2 = ps.tile([128, 512], f32, tag="p2")
                    nc.tensor.matmul(p2[:], xq0[:, qs], xk0[:, ks], start=True, stop=False)
                    nc.tensor.matmul(p2[:], xq1[:, qs], xk1[:, ks], start=False, stop=True)
                    p1 = ps.tile([128, 512], f32, tag="p1")
                    nc.tensor.matmul(p1[:], cq[:, qs], ck[:, ks], start=True, stop=True)

                    # ACT: ux, dx(-ux) from P2; uc from P1 with row-sum accum (B)
                    nc.scalar.copy(ux_t[:, ks], p2[:])
                    nc.scalar.activation(dx_t[:, ks], p2[:], AF.Copy, bias=0.0, scale=-1.0)
                    uc_c = chunk.tile([128, 512], f32, tag="uc")
                    nc.scalar.activation(
                        uc_c[:], p1[:], AF.Copy, bias=0.0, scale=1.0,
                        accum_out=bparts[:, kc:kc + 1],
                    )
                    # GPSIMD: m = ux + uc; t = uc*ux (for the loss row-sums)
                    nc.gpsimd.tensor_tensor(m_t[:, ks], ux_t[:, ks], uc_c[:], op=OP.add)
                    scr = chunk.tile([128, 512], f32, tag="scr")
                    nc.gpsimd.tensor_tensor(scr[:], ux_t[:, ks], uc_c[:], op=OP.mult)
                    # DVE: A partial = sum_j uc*ux
                    nc.vector.tensor_reduce(
                        aparts[:, kc:kc + 1], scr[:],
                        axis=mybir.AxisListType.X, op=OP.add)

                outt = small.tile([128, OUTW], f32, tag="outt")
                # layout: 0:8 IK(u32) | 8:10 ICF(u32) | 10:11 A | 11:12 B |
                #         12:36 V(f32) | 36:48 zeros
                # ---- fused top-k: per-window top-8 candidates, merge, stride-2 index
                cm = small.tile([128, NWIN * 8], f32, tag="cm")
                for w in range(NWIN):
                    nc.vector.max(cm[:, w * 8:(w + 1) * 8], m_t[:, w * W:(w + 1) * W])
                nc.vector.max(outt[:, 12:20], cm[:])                      # ranks 0-7
                nc.vector.match_replace(cm[:], outt[:, 12:20], cm[:], -1.0e30)
                nc.vector.max(outt[:, 20:28], cm[:])                      # ranks 8-15
                nc.vector.match_replace(cm[:], outt[:, 20:28], cm[:], -1.0e30)
                nc.vector.max(outt[:, 28:36], cm[:])                      # ranks 16-23
                # indices of ranks 2,4,...,16 == V cols 14,16,...,28
                nc.vector.max_index(
                    outt[:, 0:8].bitcast(u32), outt[:, 14:30:2], m_t[:])

                # ---- closest (rank-1 of ux max8; rank-0 is the diagonal) and
                # ---- farthest (rank-0 of (-ux) max8), indexed in ux
                vx = small.tile([128, 8], f32, tag="vx")
                nc.vector.max(vx[:], ux_t[:])
                vf = small.tile([128, 8], f32, tag="vf")
                nc.vector.max(vf[:], dx_t[:])
                w2 = small.tile([128, 8], f32, tag="w2")
                nc.vector.memset(w2[:, 2:8], -1.0e30)
                nc.vector.tensor_copy(w2[:, 0:1], vx[:, 1:2])
                nc.vector.tensor_scalar_mul(w2[:, 1:2], vf[:, 0:1], -1.0)
                icf_scr = small.tile([128, 8], u32, tag="icf")
                nc.vector.max_index(icf_scr[:], w2[:], ux_t[:])
                nc.vector.tensor_copy(outt[:, 8:10].bitcast(u32), icf_scr[:, 0:2])

                # ---- A, B row sums
                nc.vector.tensor_reduce(
                    outt[:, 10:11], aparts[:], axis=mybir.AxisListType.X, op=OP.add)
                nc.vector.tensor_reduce(
                    outt[:, 11:12], bparts[:], axis=mybir.AxisListType.X, op=OP.add)

                nc.vector.tensor_copy(outt[:, 36:44], aparts[:])   # debug
                nc.vector.tensor_copy(outt[:, 44:48], bparts[:, 0:4])  # debug
                nc.sync.dma_start(out_d[qs, :], outt[:])

    nc.compile()
    return nc


def _get_program():
    global _PROGRAM
    if _PROGRAM is None:
        _PROGRAM = _build_program()
    return _PROGRAM


def _host_prep(xb, cb):
    """Exact f32 prep for one batch: normalized features + sumsq rows."""
    nx = np.sqrt((xb * xb).sum(axis=0))
    xn = xb / np.maximum(nx, EPS)
    ncn = np.sqrt((cb * cb).sum(axis=0))
    cn = cb / np.maximum(ncn, EPS)
    ax = (xn * xn).sum(axis=0).astype(np.float32)
    ac = (cn * cn).sum(axis=0).astype(np.float32)
    return xn, cn, ax, ac


def _in_map(xn, cn, ax, ac, r0):
    ones_k = np.ones((1, N), np.float32)
    q = slice(r0, r0 + ROWS_PER_CORE)
    neg1_q = np.full((1, ROWS_PER_CORE), -1.0, np.float32)
    return {
        "xk0": np.ascontiguousarray(xn[0:128]),
        "xk1": np.ascontiguousarray(np.concatenate([xn[128:192], ones_k, ax[None]], 0)),
        "ck": np.ascontiguousarray(np.concatenate([cn, ones_k, ac[None]], 0)),
        "xq0": np.ascontiguousarray(2.0 * xn[0:128, q]),
        "xq1": np.ascontiguousarray(
            np.concatenate([2.0 * xn[128:192, q], -ax[None, q], neg1_q], 0)),
        "cq": np.ascontiguousarray(
            np.concatenate([2.0 * cn[:, q], -ac[None, q], neg1_q], 0)),
    }


def kernel(x, code):
    from concourse.bass_utils import run_bass_kernel_spmd

    x = np.asarray(x)
    code = np.asarray(code)
    nc = _get_program()

    in_maps = []
    for b in range(B):
        xn, cn, ax, ac = _host_prep(
            x[b, :, :, 0].astype(np.float32), code[b, :, :, 0].astype(np.float32))
        for rb in range(4):
            in_maps.append(_in_map(xn, cn, ax, ac, rb * ROWS_PER_CORE))

    res = run_bass_kernel_spmd(nc, in_maps, core_ids=list(range(8)))

    edge_nn = np.empty((B, N, 9), np.int32)
    closest = np.empty((B, N), np.int64)
    farthest = np.empty((B, N), np.int64)
    A = np.empty((B, N), np.float64)
    Bv = np.empty((B, N), np.float64)
    for ci in range(8):
        b, rb = divmod(ci, 4)
        r0 = rb * ROWS_PER_CORE
        o = res.results[ci]["out"]
        sl = slice(r0, r0 + ROWS_PER_CORE)
        ik = o[:, 0:8].view(np.uint32).astype(np.int32)
        icf = o[:, 8:10].view(np.uint32)
        edge_nn[b, sl, 0] = np.arange(r0, r0 + ROWS_PER_CORE, dtype=np.int32)
        edge_nn[b, sl, 1:] = ik
        closest[b, sl] = icf[:, 0]
        farthest[b, sl] = icf[:, 1]
        A[b, sl] = o[:, 10].astype(np.float64)
        Bv[b, sl] = -o[:, 11].astype(np.float64)

    tot = 0.0
    for b in range(B):
        s_intra = (-A[b] + 0.12 * Bv[b]).sum()
        cnt_c = np.bincount(closest[b], minlength=N).astype(np.float64)
        cnt_f = np.bincount(farthest[b], minlength=N).astype(np.float64)
        s_pos = (cnt_c * (-A[b] + 0.2 * Bv[b])).sum()
        s_neg = (cnt_f * (-A[b] + 1.0 * Bv[b])).sum()
        tot += 0.1 * s_intra + 1.0 * s_pos + 0.15 * s_neg
    drl = np.float32(tot / (B * N * N))

    center = np.broadcast_to(np.arange(N, dtype=np.int32)[None, :, None], (B, N, 9))
    edge_index = np.stack([edge_nn, np.ascontiguousarray(center)], axis=0).astype(np.int32)
    return edge_index, drl


# revision 12
# speedup vs baseline: 26.4277x; 1.1684x over previous
"""Trainium2 Bass kernel for nn_DenseDilatedKnnGraph.

Contract: kernel(x, code) takes FULL inputs (numpy), returns the FULL output
(edge_index int32 (2, 2, 4096, 9), drl f32 scalar) exactly like the jax
reference.

Sharding: 8 cores = 2 batches x 4 row-blocks of 1024 query rows. Every core
receives the full key set of its batch plus its query slice (prepared on the
host), computes its 1024x4096 similarity tiles on the PE, and reduces to
per-row outputs (top-k indices, argmin/argmax, loss row-sums) on-chip. Only
O(N) data ever leaves the device.

Math notes (u = "negated distance before relu"):
  ux[i,j] = 2*Sx - ax_i - ax_j   (dist_x = relu(-ux))
  uc[i,j] = 2*Sc - ac_i - ac_j   (dist_c = relu(-uc))
  m = ux + uc                    (fused ranking value; top-k of dist = max8 of m)
  A[i] = sum_j uc*ux  ~= sum_j max(dist_c,1e-5)*dist_x   (loss row-sums; the
         1e-5 clamp and relu only matter on the diagonal, ~1e-12 effect)
  B[i] = -sum_j uc    ~= sum_j max(dist_c,1e-5)
The distance matrices are symmetric, so row sums equal the column sums the
reference's gathered-matrix means reduce to.
"""
import sys

if "/opt/trn_rl_repo" not in sys.path:
    sys.path.insert(0, "/opt/trn_rl_repo")

import numpy as np

B = 2
N = 4096
CX = 192
CC = 64
ROWS_PER_CORE = 1024
QT = ROWS_PER_CORE // 128        # query tiles per core
KC = N // 512                    # key chunks per row tile
W = 256                          # top-k candidate window
NWIN = N // W                    # 16 windows -> 128 candidates
OUTW = 48                        # packed output width per row
EPS = np.float32(1e-12)

_PROGRAM = None


def _build_program(reps=1, dummy_reps=0):
    """Build the SPMD program. reps>1 repeats the compute (for timing);
    dummy_reps appends structurally-identical iterations with tiny access
    patterns (same instruction count, ~zero data) so NEFF-size-dependent
    dispatch overheads cancel in wall-clock deltas."""
    import concourse.mybir as mybir
    import concourse.tile as tile
    from concourse import bacc

    f32 = mybir.dt.float32
    u32 = mybir.dt.uint32
    AF = mybir.ActivationFunctionType
    OP = mybir.AluOpType

    nc = bacc.Bacc(None, target_bir_lowering=False, debug=False)

    xk0_d = nc.dram_tensor("xk0", [128, N], f32, kind="ExternalInput").ap()
    xk1_d = nc.dram_tensor("xk1", [66, N], f32, kind="ExternalInput").ap()
    ck_d = nc.dram_tensor("ck", [66, N], f32, kind="ExternalInput").ap()
    xq0_d = nc.dram_tensor("xq0", [128, ROWS_PER_CORE], f32, kind="ExternalInput").ap()
    xq1_d = nc.dram_tensor("xq1", [66, ROWS_PER_CORE], f32, kind="ExternalInput").ap()
    cq_d = nc.dram_tensor("cq", [66, ROWS_PER_CORE], f32, kind="ExternalInput").ap()
    out_d = nc.dram_tensor("out", [ROWS_PER_CORE, OUTW], f32, kind="ExternalOutput").ap()

    with tile.TileContext(nc) as tc:
        with (
            tc.tile_pool(name="inp", bufs=1) as inp,
            tc.tile_pool(name="rows", bufs=2) as rows,
            tc.tile_pool(name="chunk", bufs=3) as chunk,
            tc.tile_pool(name="small", bufs=2) as small,
            tc.tile_pool(name="ps", bufs=2, space="PSUM") as ps,
        ):
            xk0 = inp.tile([128, N], f32)
            xk1 = inp.tile([66, N], f32)
            ck = inp.tile([66, N], f32)
            xq0 = inp.tile([128, ROWS_PER_CORE], f32)
            xq1 = inp.tile([66, ROWS_PER_CORE], f32)
            cq = inp.tile([66, ROWS_PER_CORE], f32)
            nc.sync.dma_start(xk0[:], xk0_d[:])
            nc.sync.dma_start(xk1[:], xk1_d[:])
            nc.sync.dma_start(ck[:], ck_d[:])
            nc.sync.dma_start(xq0[:], xq0_d[:])
            nc.sync.dma_start(xq1[:], xq1_d[:])
            nc.sync.dma_start(cq[:], cq_d[:])

            for qt_rep in range(QT * (reps + dummy_reps)):
                qt = qt_rep % QT
                dummy = qt_rep >= QT * reps
                qs = slice(qt * 128, (qt + 1) * 128)
                CW = 8 if dummy else 512        # key-chunk width
                MW = 8 if dummy else W          # candidate window width
                FR = 8 if dummy else N          # "full row" width
                ux_t = rows.tile([128, N], f32, tag="ux")
                dx_t = rows.tile([128, N], f32, tag="dx")
                m_t = rows.tile([128, N], f32, tag="m")
                aparts = small.tile([128, KC], f32, tag="aparts")
                bparts = small.tile([128, KC], f32, tag="bparts")

                for kc in range(KC):
                    ks = slice(kc * CW, (kc + 1) * CW)
                    p2 = ps.tile([128, 512], f32, tag="p2")
                    nc.tensor.matmul(p2[:, 0:CW], xq0[:, qs], xk0[:, ks], start=True, stop=False)
                    nc.tensor.matmul(p2[:, 0:CW], xq1[:, qs], xk1[:, ks], start=False, stop=True)
                    p1 = ps.tile([128, 512], f32, tag="p1")
                    nc.tensor.matmul(p1[:, 0:CW], cq[:, qs], ck[:, ks], start=True, stop=True)

                    # ACT: ux, dx(-ux) from P2; uc from P1 with row-sum accum (B)
                    nc.scalar.copy(ux_t[:, ks], p2[:, 0:CW])
                    nc.scalar.activation(dx_t[:, ks], p2[:, 0:CW], AF.Copy, bias=0.0, scale=-1.0)
                    uc_c = chunk.tile([128, 512], f32, tag="uc")
                    nc.scalar.activation(
                        uc_c[:, 0:CW], p1[:, 0:CW], AF.Copy, bias=0.0, scale=1.0,
                        accum_out=bparts[:, kc:kc + 1],
                    )
                    # GPSIMD: m = ux + uc; t = uc*ux (for the loss row-sums)
                    nc.gpsimd.tensor_tensor(m_t[:, ks], ux_t[:, ks], uc_c[:, 0:CW], op=OP.add)
                    scr = chunk.tile([128, 512], f32, tag="scr")
                    nc.gpsimd.tensor_tensor(scr[:, 0:CW], ux_t[:, ks], uc_c[:, 0:CW], op=OP.mult)
                    # DVE: A partial = sum_j uc*ux
                    nc.vector.tensor_reduce(
                        aparts[:, kc:kc + 1], scr[:, 0:CW],
                        axis=mybir.AxisListType.X, op=OP.add)

                outt = small.tile([128, OUTW], f32, tag="outt")
                # layout: 0:8 IK(u32) | 8:10 ICF(u32) | 10:11 A | 11:12 B |
                #         12:36 V(f32) | 36:48 debug
                # ---- fused top-k: per-window top-8 candidates, merge, stride-2 index
                cm = small.tile([128, NWIN * 8], f32, tag="cm")
                for w in range(NWIN):
                    nc.vector.max(cm[:, w * 8:(w + 1) * 8], m_t[:, w * MW:w * MW + MW])
                nc.vector.max(outt[:, 12:20], cm[:])                      # ranks 0-7
                nc.vector.match_replace(cm[:], outt[:, 12:20], cm[:], -1.0e30)
                nc.vector.max(outt[:, 20:28], cm[:])                      # ranks 8-15
                nc.vector.match_replace(cm[:], outt[:, 20:28], cm[:], -1.0e30)
                nc.vector.max(outt[:, 28:36], cm[:])                      # ranks 16-23
                # indices of ranks 2,4,...,16 == V cols 14,16,...,28
                nc.vector.max_index(
                    outt[:, 0:8].bitcast(u32), outt[:, 14:30:2], m_t[:, 0:FR])

                # ---- closest (rank-1 of ux max8; rank-0 is the diagonal) and
                # ---- farthest (rank-0 of (-ux) max8), indexed in ux
                vx = small.tile([128, 8], f32, tag="vx")
                nc.vector.max(vx[:], ux_t[:, 0:FR])
                vf = small.tile([128, 8], f32, tag="vf")
                nc.vector.max(vf[:], dx_t[:, 0:FR])
                w2 = small.tile([128, 8], f32, tag="w2")
                nc.vector.memset(w2[:, 2:8], -1.0e30)
                nc.vector.tensor_copy(w2[:, 0:1], vx[:, 1:2])
                nc.vector.tensor_scalar_mul(w2[:, 1:2], vf[:, 0:1], -1.0)
                icf_scr = small.tile([128, 8], u32, tag="icf")
                nc.vector.max_index(icf_scr[:], w2[:], ux_t[:, 0:FR])
                nc.vector.tensor_copy(outt[:, 8:10].bitcast(u32), icf_scr[:, 0:2])

                # ---- A, B row sums
                nc.vector.tensor_reduce(
                    outt[:, 10:11], aparts[:], axis=mybir.AxisListType.X, op=OP.add)
                nc.vector.tensor_reduce(
                    outt[:, 11:12], bparts[:], axis=mybir.AxisListType.X, op=OP.add)

                nc.vector.tensor_copy(outt[:, 36:44], aparts[:])   # debug
                nc.vector.tensor_copy(outt[:, 44:48], bparts[:, 0:4])  # debug
                if not dummy:
                    nc.sync.dma_start(out_d[qs, :], outt[:])

    nc.compile()
    return nc


def _get_program():
    global _PROGRAM
    if _PROGRAM is None:
        _PROGRAM = _build_program()
    return _PROGRAM


def _host_prep(xb, cb):
    """Exact f32 prep for one batch: normalized features + sumsq rows."""
    nx = np.sqrt((xb * xb).sum(axis=0))
    xn = xb / np.maximum(nx, EPS)
    ncn = np.sqrt((cb * cb).sum(axis=0))
    cn = cb / np.maximum(ncn, EPS)
    ax = (xn * xn).sum(axis=0).astype(np.float32)
    ac = (cn * cn).sum(axis=0).astype(np.float32)
    return xn, cn, ax, ac


def _in_map(xn, cn, ax, ac, r0):
    ones_k = np.ones((1, N), np.float32)
    q = slice(r0, r0 + ROWS_PER_CORE)
    neg1_q = np.full((1, ROWS_PER_CORE), -1.0, np.float32)
    return {
        "xk0": np.ascontiguousarray(xn[0:128]),
        "xk1": np.ascontiguousarray(np.concatenate([xn[128:192], ones_k, ax[None]], 0)),
        "ck": np.ascontiguousarray(np.concatenate([cn, ones_k, ac[None]], 0)),
        "xq0": np.ascontiguousarray(2.0 * xn[0:128, q]),
        "xq1": np.ascontiguousarray(
            np.concatenate([2.0 * xn[128:192, q], -ax[None, q], neg1_q], 0)),
        "cq": np.ascontiguousarray(
            np.concatenate([2.0 * cn[:, q], -ac[None, q], neg1_q], 0)),
    }


def kernel(x, code):
    from concourse.bass_utils import run_bass_kernel_spmd

    x = np.asarray(x)
    code = np.asarray(code)
    nc = _get_program()

    in_maps = []
    for b in range(B):
        xn, cn, ax, ac = _host_prep(
            x[b, :, :, 0].astype(np.float32), code[b, :, :, 0].astype(np.float32))
        for rb in range(4):
            in_maps.append(_in_map(xn, cn, ax, ac, rb * ROWS_PER_CORE))

    res = run_bass_kernel_spmd(nc, in_maps, core_ids=list(range(8)))

    edge_nn = np.empty((B, N, 9), np.int32)
    closest = np.empty((B, N), np.int64)
    farthest = np.empty((B, N), np.int64)
    A = np.empty((B, N), np.float64)
    Bv = np.empty((B, N), np.float64)
    for ci in range(8):
        b, rb = divmod(ci, 4)
        r0 = rb * ROWS_PER_CORE
        o = res.results[ci]["out"]
        sl = slice(r0, r0 + ROWS_PER_CORE)
        ik = o[:, 0:8].view(np.uint32).astype(np.int32)
        icf = o[:, 8:10].view(np.uint32)
        edge_nn[b, sl, 0] = np.arange(r0, r0 + ROWS_PER_CORE, dtype=np.int32)
        edge_nn[b, sl, 1:] = ik
        closest[b, sl] = icf[:, 0]
        farthest[b, sl] = icf[:, 1]
        A[b, sl] = o[:, 10].astype(np.float64)
        Bv[b, sl] = -o[:, 11].astype(np.float64)

    tot = 0.0
    for b in range(B):
        s_intra = (-A[b] + 0.12 * Bv[b]).sum()
        cnt_c = np.bincount(closest[b], minlength=N).astype(np.float64)
        cnt_f = np.bincount(farthest[b], minlength=N).astype(np.float64)
        s_pos = (cnt_c * (-A[b] + 0.2 * Bv[b])).sum()
        s_neg = (cnt_f * (-A[b] + 1.0 * Bv[b])).sum()
        tot += 0.1 * s_intra + 1.0 * s_pos + 0.15 * s_neg
    drl = np.float32(tot / (B * N * N))

    center = np.broadcast_to(np.arange(N, dtype=np.int32)[None, :, None], (B, N, 9))
    edge_index = np.stack([edge_nn, np.ascontiguousarray(center)], axis=0).astype(np.int32)
    return edge_index, drl
